# revision 13
# baseline (speedup 1.0000x reference)
"""TRN2 Bass kernel for nn_Brain: delayed-synapse recurrent network.

Strategy (banded delay-batched futures, v2):
  total_input[t] = c0 + sum_{d=1}^{15} W_d @ acts_{t-d};  acts_t = tanh(...)
- Edges with delay >= 16 never fire; delay-0 edges give a per-neuron
  constant c0 (host-computed).
- Neurons are SPATIALLY SORTED: delays are int(Euclidean distance) of
  points in a cube, so graph distances (Dijkstra with edge length d+0.5)
  recover a 1D landmark projection; sorting by it makes every W_d BANDED
  (|rank_src - rank_tgt| bounded ~ (d+1)/chunk_width). Out-of-band
  128x128 weight tiles are skipped -> ~2.3x fewer matmul+ldweights pairs
  (the kernel is PE-instruction bound).
- Targets sharded contiguously in sorted order (512/core). SPMD identical
  program: per-core band windows are made uniform by keeping the
  activation history in CORE-ROTATED chunk coordinates: rotated chunk c
  holds global chunk (4k + c) mod 32 on core k. The per-step AllGather
  lands via a DynSlice (partition_id-offset) DMA window into a 3x-copied
  DRAM bounce buffer; the doubled (64-chunk) rotated history makes all
  band windows static program slices.
- Bucket d is applied in ceil((16-d)/d) windows of nb<=d steps (both
  batch rows ride as extra matmul columns). d<=11 banded weights stay
  SBUF-resident; d=12..15 are full-width and stream with a 3-buf pool.
- Work is split into (app, target-chunk) pieces, greedily load-balanced
  across steps within [ready, deadline] so the PE stays busy during the
  collectives without blocking each step's critical d=1 chain.
"""
import numpy as np

N_NEURONS = 4096
INPUT_SIZE = 1024
BATCH = 2
STEPS = 16
N_CORES = 8
TGT_PER_CORE = N_NEURONS // N_CORES        # 512
TCH = TGT_PER_CORE // 128                  # 4 target chunks per core
SCH = N_NEURONS // 128                     # 32 global source chunks
MAXD = STEPS - 1                           # delays 1..15 useful
RESIDENT_D = tuple(range(1, 12))           # d=1..11 resident
STREAM_D = tuple(range(12, 16))            # d=12..15 streamed
FP8_SCALE = 64.0
MAXB = 8

_compiled = None
_compiled_key = None


def _schedule():
    """Apps: (d, s0, nb) -> contributes to steps t in [s0+d, s0+d+nb-1]
    using acts_{s0..s0+nb-1} (nb <= d, windows balanced per bucket)."""
    apps = []
    for d in range(1, MAXD + 1):
        nsteps = STEPS - d
        nwin = -(-nsteps // d)
        base, extra = divmod(nsteps, nwin)
        s0 = 1
        for i in range(nwin):
            nb = base + (1 if i < extra else 0)
            apps.append((d, s0, nb))
            s0 += nb
    return apps


def _make_plan(delay_values, connection_indices, rank):
    """Band windows per (d, tc) + greedy piece schedule. Core-independent."""
    dl = np.asarray(delay_values)
    ci = np.asarray(connection_indices)
    gs = rank[ci[0].astype(np.int64)] // 128
    gt = rank[ci[1].astype(np.int64)] // 128

    # window (delta_min, width) per (d, tc): signed source-chunk offset
    # relative to the target's global chunk, max span over all cores
    win = {}
    for d in range(1, MAXD + 1):
        m = dl == d
        gsd, gtd = gs[m], gt[m]
        tcd = gtd % 4
        for tc in range(TCH):
            mm = tcd == tc
            if not mm.any():
                win[(d, tc)] = None
                continue
            delta = ((gsd[mm] - gtd[mm] + 16) % 32) - 16
            dmin, dmax = int(delta.min()), int(delta.max())
            if dmax - dmin + 1 >= SCH:
                dmin, dmax = -16, 15
            win[(d, tc)] = (dmin, dmax - dmin + 1)

    # pieces: (d, s0, nb, tc) with ready/deadline; greedy least-loaded step
    apps = _schedule()
    pieces = []
    for (d, s0, nb) in apps:
        for tc in range(TCH):
            if win[(d, tc)] is None:
                continue
            pieces.append((d, s0, nb, tc))
    load = {t: 0 for t in range(1, STEPS)}
    assign = {t: [] for t in range(1, STEPS)}
    # forced pieces first (smaller slack first), then big ones
    pieces.sort(key=lambda p: ((p[1] + p[0] - 1) - (p[1] + p[2] - 1),
                               -win[(p[0], p[3])][1]))
    for (d, s0, nb, tc) in pieces:
        ready, deadline = s0 + nb - 1, s0 + d - 1
        cost = win[(d, tc)][1]
        t = min(range(ready, deadline + 1), key=lambda x: load[x])
        load[t] += cost
        assign[t].append((d, s0, nb, tc))
    for t in assign:
        # pieces not needing this step's fresh acts go first: they keep
        # the in-order PE busy during the collective+landing
        assign[t].sort(key=lambda p, tt=t: (p[1] + p[2] - 1 == tt,
                                            p[1] + p[0] - 1, p[0]))
    # weight-load issue step per deferred bucket (d>=4 load lazily)
    first_use = {d: min((t for t in assign for p in assign[t] if p[0] == d),
                        default=1) for d in range(4, MAXD + 1)}
    return win, assign, first_use


def _build_program(plan):
    from concourse import bacc, mybir, tile
    from concourse.bass import ds

    win, assign, first_use = plan
    dt = mybir.dt
    nc = bacc.Bacc(None, target_bir_lowering=False, debug=False)

    # per-d packed banded weight columns: (tc-major, slot, 128)
    wcols = {d: sum(win[(d, tc)][1] for tc in range(TCH)
                    if win[(d, tc)] is not None) * 128
             for d in range(1, MAXD + 1)}
    colbase = {}
    for d in range(1, MAXD + 1):
        c = 0
        for tc in range(TCH):
            colbase[(d, tc)] = c
            if win[(d, tc)] is not None:
                c += win[(d, tc)][1] * 128

    wd_in = {d: nc.declare_dram_parameter(f"wd{d}", [128, wcols[d]],
                                          dt.float8e4, isOutput=False)
             for d in range(1, MAXD + 1)}
    c0r_in = nc.declare_dram_parameter("c0rep", [128, TCH * STEPS * BATCH],
                                       dt.float32, isOutput=False)
    out_d = nc.declare_dram_parameter("out", [128, TCH * BATCH], dt.float32,
                                      isOutput=True)

    # collective bounce: rows are (partition, tc)-major within each core
    # block so the rotated landing moves contiguous 16B runs; cc2 has 3
    # stacked copies so the per-core 64-chunk window never needs a modulo.
    # cin8 replicates own acts 8x so a single AllToAll (one-phase pairwise
    # exchange, lower latency than ring AllGather) produces the same
    # rank-concatenated result.
    cin8 = nc.dram_tensor("cc_in8", [N_NEURONS, BATCH], dt.bfloat16)
    cmid = nc.dram_tensor("cc_mid", [N_NEURONS, BATCH], dt.bfloat16)
    cc2 = nc.dram_tensor("cc2", [3 * N_NEURONS, BATCH], dt.bfloat16)

    HC = 2 * SCH                                # doubled rotated chunks

    with tile.TileContext(nc) as tc_ctx:
        with (
            tc_ctx.tile_pool(name="wres", bufs=1) as wres_pool,
            tc_ctx.tile_pool(name="wstream", bufs=4) as wstream_pool,
            tc_ctx.tile_pool(name="aux", bufs=1) as aux_pool,
            tc_ctx.tile_pool(name="psum", bufs=4, space="PSUM") as psum_pool,
        ):
            t_wres = {d: wres_pool.tile([128, wcols[d]], dt.float8e4,
                                        name=f"wres{d}", tag=f"wres{d}")
                      for d in RESIDENT_D}
            t_wstr = {}
            t_acc = aux_pool.tile([128, TCH * STEPS * BATCH], dt.float32)
            t_hist = aux_pool.tile([128, MAXD * HC * BATCH], dt.bfloat16)
            t_actb = aux_pool.tile([128, TCH * BATCH], dt.bfloat16)
            t_act = aux_pool.tile([128, TCH * BATCH], dt.float32)

            nc.sync.dma_start(t_acc[:], c0r_in[:])
            for d in RESIDENT_D:
                if d < 4:
                    nc.scalar.dma_start(t_wres[d][:], wd_in[d][:])

            off_eng = {}
            for eng in (nc.sync, nc.scalar, nc.gpsimd):
                off_eng[eng] = eng.partition_id() * TGT_PER_CORE

            hist4 = t_hist[:].rearrange("p (s c r) -> p s c r",
                                        s=MAXD, c=HC)
            acc4 = t_acc[:].rearrange("p (tc t r) -> p tc t r",
                                      tc=TCH, t=STEPS)

            def run_piece(d, s0, nb, tc):
                dmin, W = win[(d, tc)]
                st = tc + SCH + dmin            # rotated doubled slot base
                t_w = t_wres[d] if d in RESIDENT_D else t_wstr[d]
                t_scr = psum_pool.tile([128, MAXB * BATCH], dt.float32,
                                       name="scr", tag="scr")
                scr = t_scr[:].rearrange("p (b r) -> p b r", r=BATCH)
                cb = colbase[(d, tc)]
                for i in range(W):
                    lhsT = t_w[:, cb + i * 128: cb + (i + 1) * 128]
                    rhs = hist4[:, s0 - 1:s0 - 1 + nb, st + i, :]
                    nc.tensor.matmul(scr[:, :nb, :], lhsT, rhs,
                                     start=(i == 0), stop=(i == W - 1))
                t0 = s0 + d
                acc_win = acc4[:, tc, t0 - 1:t0 - 1 + nb, :]
                nc.vector.scalar_tensor_tensor(
                    acc_win, scr[:, :nb, :], 1.0 / FP8_SCALE, acc_win,
                    mybir.AluOpType.mult, mybir.AluOpType.add)

            for t in range(1, STEPS + 1):
                sc_ctx = nc.named_scope(f"step{t:02d}")
                sc_ctx.__enter__()
                acc_t = acc4[:, :, t - 1, :]
                if t == STEPS:
                    nc.scalar.activation(
                        t_act[:].rearrange("p (tc r) -> p tc r", tc=TCH),
                        acc_t, mybir.ActivationFunctionType.Tanh)
                    nc.sync.dma_start(out_d[:], t_act[:])
                    sc_ctx.__exit__(None, None, None)
                    break
                nc.scalar.activation(
                    t_actb[:].rearrange("p (tc r) -> p tc r", tc=TCH),
                    acc_t, mybir.ActivationFunctionType.Tanh)
                # own acts replicated 8x -> one-phase AllToAll -> tripled
                # copies (parallel queues) -> rotated 16B-run landing
                # (split across 3 queues)
                nc.sync.dma_start(
                    cin8[:].rearrange("(j p f) r -> p j (f r)",
                                      p=128, f=TCH),
                    t_actb[:].unsqueeze(1).broadcast_to(
                        (128, N_CORES, TCH * BATCH)))
                nc.gpsimd.collective_compute(
                    "AllToAll", mybir.AluOpType.bypass,
                    replica_groups=[list(range(N_CORES))],
                    ins=[cin8[:]], outs=[cmid[:]])
                nc.sync.dma_start(cc2[0:N_NEURONS, :], cmid[:])
                nc.scalar.dma_start(cc2[N_NEURONS:2 * N_NEURONS, :], cmid[:])
                nc.sync.dma_start(cc2[2 * N_NEURONS:3 * N_NEURONS, :],
                                  cmid[:])
                dst = hist4[:, t - 1, :, :].rearrange(
                    "p (j f) r -> p j (f r)", f=TCH)
                for (j0, j1, eng) in ((0, 5, nc.sync), (5, 10, nc.scalar),
                                      (10, 16, nc.gpsimd)):
                    src = cc2[ds(off_eng[eng] + j0 * TGT_PER_CORE,
                                 (j1 - j0) * TGT_PER_CORE), :].rearrange(
                        "(j p f) r -> p j (f r)", p=128, f=TCH)
                    eng.dma_start(dst[:, j0:j1, :], src)
                sc_ctx.__exit__(None, None, None)
                # deferred weight loads scheduled just before first use
                for d in sorted(first_use):
                    if first_use[d] - 2 == t or (t == 1 and first_use[d] <= 3):
                        if d in RESIDENT_D:
                            nc.scalar.dma_start(t_wres[d][:], wd_in[d][:])
                        elif d not in t_wstr:
                            t_wstr[d] = wstream_pool.tile(
                                [128, wcols[d]], dt.float8e4,
                                name="wstr", tag="wstr")
                            nc.scalar.dma_start(t_wstr[d][:], wd_in[d][:])
                for (d, s0, nb, tc) in assign[t]:
                    with nc.named_scope(f"app_d{d}_s{s0}_c{tc}"):
                        run_piece(d, s0, nb, tc)

    nc.compile()
    return nc


def _spatial_rank(connection_indices, delay_values):
    """Estimate 1D landmark projection from graph distances; return rank."""
    import scipy.sparse as sp
    from scipy.sparse.csgraph import dijkstra
    ci = np.asarray(connection_indices)
    dl = np.asarray(delay_values)
    src = ci[0].astype(np.int64)
    tgt = ci[1].astype(np.int64)
    w = dl.astype(np.float64) + 0.5
    rr = np.concatenate([src, tgt])
    cc = np.concatenate([tgt, src])
    ww = np.concatenate([w, w])
    order = np.lexsort((cc, rr))
    rr, cc, ww = rr[order], cc[order], ww[order]
    same = (rr[1:] == rr[:-1]) & (cc[1:] == cc[:-1])
    starts = np.flatnonzero(np.concatenate([[True], ~same]))
    wmin = np.minimum.reduceat(ww, starts)
    G = sp.csr_matrix((wmin, (rr[starts], cc[starts])),
                      shape=(N_NEURONS, N_NEURONS))
    D0 = dijkstra(G, indices=0)
    t1 = int(np.argmax(D0))
    D1 = dijkstra(G, indices=t1)
    t2 = int(np.argmax(D1))
    D2 = dijkstra(G, indices=t2)
    proj = (D1 ** 2 - D2 ** 2) / (2.0 * max(D1[t2], 1e-9))
    pi = np.argsort(proj, kind="stable")
    rank = np.empty(N_NEURONS, np.int64)
    rank[pi] = np.arange(N_NEURONS)
    return pi, rank


def _preprocess(input_data, connection_weights, connection_indices,
                delay_values, steps):
    """Host: permutation, banded per-core weights, c0, plan."""
    import ml_dtypes
    assert steps == STEPS
    w = np.asarray(connection_weights, np.float32)
    ci = np.asarray(connection_indices)
    dl = np.asarray(delay_values)
    x = np.asarray(input_data, np.float32)

    pi, rank = _spatial_rank(ci, dl)
    plan = _make_plan(dl, ci, rank)
    win, _, _ = plan

    src = rank[ci[0].astype(np.int64)]      # sorted coords
    tgt = rank[ci[1].astype(np.int64)]

    acts0 = np.zeros((BATCH, N_NEURONS), np.float32)
    acts0[:, :INPUT_SIZE] = x               # original order
    acts0 = acts0[:, pi]                    # -> rank order

    m0 = dl == 0
    c0 = np.zeros((BATCH, N_NEURONS), np.float32)
    for r in range(BATCH):
        np.add.at(c0[r], tgt[m0], w[m0] * acts0[r, src[m0]])

    wds = {}
    for d in range(1, MAXD + 1):
        md = dl == d
        Wd = np.zeros((N_NEURONS, N_NEURONS), np.float32)
        np.add.at(Wd, (src[md], tgt[md]), w[md])
        wds[d] = (Wd * FP8_SCALE).astype(ml_dtypes.float8_e4m3fn)

    in_maps = []
    for k in range(N_CORES):
        im = {}
        for d in range(1, MAXD + 1):
            cols = []
            for tc in range(TCH):
                if win[(d, tc)] is None:
                    continue
                dmin, W = win[(d, tc)]
                gt_glob = 4 * k + tc
                t0c = gt_glob * 128
                for i in range(W):
                    gc = (gt_glob + dmin + i) % SCH
                    cols.append(wds[d][gc * 128:(gc + 1) * 128,
                                       t0c:t0c + 128])
            Wp = np.concatenate(cols, axis=1) if cols else \
                np.zeros((128, 0), ml_dtypes.float8_e4m3fn)
            im[f"wd{d}"] = np.ascontiguousarray(Wp)
        t0 = k * TGT_PER_CORE
        c0r = np.zeros((128, TCH, STEPS, BATCH), np.float32)
        for tci in range(TCH):
            for r in range(BATCH):
                c0r[:, tci, :, r] = c0[r, t0 + tci * 128:
                                       t0 + (tci + 1) * 128][:, None]
        im["c0rep"] = c0r.reshape(128, TCH * STEPS * BATCH)
        in_maps.append(im)
    return in_maps, plan


def kernel(input_data, connection_weights, connection_indices,
           delay_values, steps):
    global _compiled, _compiled_key
    from concourse.bass_utils import run_bass_kernel_spmd

    in_maps, plan = _preprocess(input_data, connection_weights,
                                connection_indices, delay_values, int(steps))
    key = repr(plan[0])
    if _compiled is None or _compiled_key != key:
        _compiled = _build_program(plan)
        _compiled_key = key
    res = run_bass_kernel_spmd(_compiled, in_maps, list(range(N_CORES)))

    pi, _ = _spatial_rank(connection_indices, delay_values)
    out_rank = np.zeros((BATCH, N_NEURONS), np.float32)
    for k in range(N_CORES):
        o = res.results[k]["out"]
        t0 = k * TGT_PER_CORE
        for tci in range(TCH):
            for r in range(BATCH):
                out_rank[r, t0 + tci * 128: t0 + (tci + 1) * 128] = \
                    o[:, tci * BATCH + r]
    out = np.zeros((BATCH, N_NEURONS), np.float32)
    out[:, pi] = out_rank
    return out[:, -INPUT_SIZE:].astype(np.float32)
